# revision 2
# baseline (speedup 1.0000x reference)
"""Trainium2 kernel v2 for nn_ActorNetwork (8-branch graph-attention MLP).

Data-parallel over 8 cores (1024 samples each).  The neighbor mix
x_i = sum_j c~_ij z_j (per-sample coefficients) runs on the TensorEngine:
for each source branch j, matmul(lhsT = z_j sample-major chunk,
rhs = concat_i diag(c~_ij)) accumulates x feature-major in PSUM over j.
The diag matrices for a tile are built by ONE DVE tensor_tensor (2x_1P)
from a replicated-identity constant.  Feature-major z for the gate
matmuls comes from dma_start_transpose (ACT-issued, no PE/ACT copies).
LayerNorm stats via bn_stats off PSUM; relu+affine on ACT.
"""

import numpy as np

import concourse.bass as bass
import concourse.mybir as mybir
import concourse.tile as tile
from concourse.bass_utils import run_bass_kernel_spmd

B = 8192
NCORES = 8
BL = B // NCORES
P = 128
NT = BL // P
HID = [256, 128, 64]
OUT = 5
EPS = 1e-5
D0 = 4

NEIGH = np.array([[j for j in range(8) if j != i] for i in range(8)])
_G = [
    [(0, 1), (0, 2), (0, 3), (4, 0), (5, 0), (6, 0), (7, 0)],
    [(0, 1), (2, 1), (3, 1), (4, 1), (5, 1), (6, 1), (7, 1)],
    [(0, 2), (1, 2), (3, 2), (4, 2), (5, 2), (6, 2), (7, 2)],
    [(0, 3), (1, 3), (2, 3), (4, 3), (5, 3), (6, 3), (7, 3)],
    [(0, 4), (1, 4), (2, 4), (3, 4), (5, 4), (6, 4), (7, 4)],
    [(0, 5), (1, 5), (2, 5), (3, 5), (4, 5), (6, 5), (7, 5)],
    [(0, 6), (1, 6), (2, 6), (3, 6), (4, 6), (5, 6), (7, 6)],
    [(0, 7), (1, 7), (2, 7), (3, 7), (4, 7), (5, 7), (6, 7)],
]
GIDX = np.array([[r * 8 + c for (r, c) in row] for row in _G]).reshape(-1)

F32 = mybir.dt.float32
BF16 = mybir.dt.bfloat16
AF = mybir.ActivationFunctionType
ALU = mybir.AluOpType

# tail-drain split (see kernel.py of the prior session): one sync-wait per
# semaphore lane on SP nops so the final drain carries no waits.
_orig_dab = tile.TileContext._drain_and_barrier


def _patched_dab(self, tick_clock, wait_clock):
    from concourse.vector_clock import ScopedClock, VectorClock

    gc = tick_clock.global_clock
    nproc = len(list(gc))
    for proc in range(nproc):
        val = gc[proc]
        if val <= 0:
            continue
        v = VectorClock()
        v.require_at_least(proc, val)
        nop = self.nc.sync.nop(nofuse=True)
        wait_clock.add_sem_waits(nop.ins, ScopedClock({None: v}))
    self.nc.sync.drain()
    self.nc.all_engine_barrier()
    assert self.sems is not None
    popped = self.nc._tile_sem_poison_stack.pop()
    assert popped is self._sem_poison
    self.nc.clear_and_free_semaphores(list(self.sems.allocated().values()))
    self.nc.all_engine_barrier()


tile.TileContext._drain_and_barrier = _patched_dab


def _bcast_last(ap, n):
    return bass.AP(tensor=ap.tensor, offset=ap.offset, ap=[*ap.ap, [0, n]])


def _nop_read(eng, ap):
    """Engine NOP carrying a read-AP: absorbs one foreign sem lane into the
    engine's program order so the next real instruction needs <=1 wait.
    The AP must be attached before add_instruction so the tile scheduler
    sees the dependency (it is stripped again at lowering)."""
    ins = mybir.InstNoOp(
        name=eng.bass.get_next_instruction_name(),
        bass_nofuse=True,
        ins=[eng.lower_ap(ap)],
    )
    return eng.add_instruction(ins)


def build_nc():
    nc = bass.Bass()
    DIMS = [D0, 256, 128]
    ECOLS = [256, 128, 64]

    x0f_ext = nc.declare_dram_parameter("x0f", [32, BL], BF16, isOutput=False)
    gates_ext = nc.declare_dram_parameter("gates", [BL, 56], F32, isOutput=False)
    identB_ext = nc.declare_dram_parameter("identB", [P, P * 4], BF16, isOutput=False)
    wgs1_ext = nc.declare_dram_parameter("wgs1", [16, 128, 56], BF16, isOutput=False)
    wgs2_ext = nc.declare_dram_parameter("wgs2", [8, 128, 56], BF16, isOutput=False)
    w0_ext = nc.declare_dram_parameter("w0", [32, 8, 256], BF16, isOutput=False)
    w1_ext = nc.declare_dram_parameter("w1", [8, 2, 128, 128], BF16, isOutput=False)
    w2_ext = nc.declare_dram_parameter("w2", [8, 128, 64], BF16, isOutput=False)
    wo_ext = nc.declare_dram_parameter("wo", [4, 128, 16], BF16, isOutput=False)
    out_ext = nc.declare_dram_parameter("out", [BL, 40], F32, isOutput=True)

    with tile.TileContext(nc) as tc:
        with (
            tc.tile_pool(name="singles", bufs=1) as singles,
            tc.tile_pool(name="dg", bufs=1) as dgp,
            tc.tile_pool(name="pxf", bufs=1, space="PSUM") as pxf,
            tc.tile_pool(name="ph", bufs=1, space="PSUM") as php,
            tc.tile_pool(name="pg", bufs=1, space="PSUM") as pgp,
            tc.tile_pool(name="po", bufs=1, space="PSUM") as pop,
        ):
            # ---------- constants / weights ----------
            eps_t = singles.tile([P, 1], F32)
            nc.vector.memset(eps_t, EPS)

            identB = singles.tile([P, P, 4], BF16)
            nc.gpsimd.dma_start(
                out=identB.rearrange("p a b -> p (a b)"), in_=identB_ext[:, :]
            )
            wgs1_sb = singles.tile([128, 16, 56], BF16)
            nc.gpsimd.dma_start(out=wgs1_sb, in_=wgs1_ext.rearrange("k p f -> p k f"))
            wgs2_sb = singles.tile([128, 8, 56], BF16)
            nc.gpsimd.dma_start(out=wgs2_sb, in_=wgs2_ext.rearrange("k p f -> p k f"))
            w0_sb = singles.tile([32, 8, 256], BF16)
            nc.gpsimd.dma_start(out=w0_sb, in_=w0_ext.rearrange("p i f -> p i f"))
            w1_sb = singles.tile([128, 8, 2, 128], BF16)
            nc.gpsimd.dma_start(out=w1_sb, in_=w1_ext.rearrange("i k p f -> p i k f"))
            w2_sb = singles.tile([128, 8, 64], BF16)
            nc.gpsimd.dma_start(out=w2_sb, in_=w2_ext.rearrange("i p f -> p i f"))
            wo_sb = singles.tile([128, 4, 16], BF16)
            nc.gpsimd.dma_start(out=wo_sb, in_=wo_ext.rearrange("c p f -> p c f"))

            xf0 = singles.tile([32, NT, P], BF16)
            nc.gpsimd.dma_start(
                out=xf0, in_=x0f_ext.rearrange("f (t p) -> f t p", p=P)
            )
            gates_sb = singles.tile([P, NT, 56], F32)
            nc.gpsimd.dma_start(
                out=gates_sb, in_=gates_ext.rearrange("(t p) f -> p t f", p=P)
            )

            # ---------- activations ----------
            z1 = singles.tile([P, NT, 8, 256], BF16)
            z2 = singles.tile([P, NT, 8, 128], BF16)
            z3 = singles.tile([P, NT, 8, 64], BF16)
            zf1 = singles.tile([P, NT, 16, P], BF16)
            zf2 = singles.tile([P, NT, 8, P], BF16)
            z3f = singles.tile([P, NT, 4, P], BF16)
            xfsb = singles.tile([P, 2, 2, 4, P], BF16)   # [chunk, half, i4, q]

            e_sb = singles.tile([P, NT, 56], F32)
            cc4 = singles.tile([P, NT, 8, 7], BF16)
            cfull0 = singles.tile([P, NT, 8, 8], BF16)
            cfull1 = singles.tile([P, NT, 8, 8], BF16)
            cfull = [cfull0, cfull1]
            s8 = singles.tile([P, NT, 8], F32)
            sr8 = singles.tile([P, NT, 8], F32)
            stats = singles.tile([P, NT, 8, 6], F32)
            t1v = singles.tile([P, NT, 8], F32)
            t2v = singles.tile([P, NT, 8], F32)
            t3v = singles.tile([P, NT, 8], F32)
            varv = singles.tile([P, NT, 8], F32)
            stdv = singles.tile([P, NT, 8], F32)
            rstd = singles.tile([P, NT, 8], F32)
            nmr = singles.tile([P, NT, 8], F32)
            scr_a = singles.tile([2, 2], F32)
            osb = singles.tile([P, NT, 64], F32)
            out_sb = singles.tile([P, NT, 40], F32)

            for l in range(2):
                nc.vector.memset(cfull[l], 1.0)

            def ln_relu(l, t, h_aps, z_out, E):
                """h_aps: list of 8 psum APs [P, E] (branch-major). bn_stats
                + baseline LN chain + 8 ACT relus into z_out[:, t]."""
                for i in range(8):
                    nc.vector.bn_stats(stats[:, t, i], h_aps[i])
                sA = stats[:, t, :, 1]
                sB = stats[:, t, :, 4]
                sC = stats[:, t, :, 2]
                sD = stats[:, t, :, 5]
                nc.vector.tensor_tensor(t1v[:, t], sA, sB, op=ALU.add)
                nc.vector.tensor_tensor(t2v[:, t], sC, sD, op=ALU.add)
                nc.vector.tensor_tensor(t3v[:, t], sA, sB, op=ALU.subtract)
                nc.vector.tensor_tensor(t3v[:, t], t3v[:, t], t3v[:, t], op=ALU.mult)
                nc.vector.tensor_scalar_mul(t2v[:, t], t2v[:, t], 1.0 / E)
                nc.vector.scalar_tensor_tensor(
                    varv[:, t], t3v[:, t], 0.25, t2v[:, t],
                    op0=ALU.mult, op1=ALU.add,
                )
                nc.scalar.activation(
                    stdv[:, t], varv[:, t], AF.Sqrt, bias=eps_t[:, 0:1]
                )
                nc.vector.reciprocal(rstd[:, t], stdv[:, t])
                nc.vector.scalar_tensor_tensor(
                    nmr[:, t], t1v[:, t], -0.5, rstd[:, t],
                    op0=ALU.mult, op1=ALU.mult,
                )
                nc.scalar.activation(scr_a, nmr[0:2, t, 0:2], AF.Copy)
                for i in range(8):
                    nc.scalar.activation(
                        z_out[:, t, i],
                        h_aps[i],
                        AF.Relu,
                        bias=nmr[:, t, i : i + 1],
                        scale=rstd[:, t, i : i + 1],
                    )

            def coeff_prep(l, pg_ps):
                """exp + softmax + gate scaling + scatter into cfull[l-1]."""
                # absorb prior-layer DVE reads of e_sb into ACT, and prior
                # gpsimd cfull writes into DVE, so Exp/TT carry <=1 wait
                if l == 2:
                    _nop_read(nc.scalar, cc4[0:2, 0, 0, 0:2])
                    _nop_read(nc.vector, cfull[0][0:2, 0, 0, 0:2])
                nc.scalar.activation(e_sb, pg_ps, AF.Exp)
                e4 = e_sb.rearrange("p t (a b) -> p t a b", a=8)
                nc.vector.tensor_reduce(
                    s8, e4, axis=mybir.AxisListType.X, op=ALU.add
                )
                nc.vector.reciprocal(
                    sr8.rearrange("p t a -> p (t a)"),
                    s8.rearrange("p t a -> p (t a)"),
                )
                # cc4 = e * gates * (1/s)  (bf16 out on the second op)
                nc.vector.tensor_tensor(e_sb, e_sb, gates_sb, op=ALU.mult)
                nc.vector.tensor_tensor(
                    cc4,
                    e_sb.rearrange("p t (a b) -> p t a b", a=8),
                    _bcast_last(sr8, 7),
                    op=ALU.mult,
                )
                # scatter (i,k) -> cfull[p, t, j, i] on gpsimd (14 rects)
                cf = cfull[l - 1]
                for k in range(7):
                    if k + 1 >= 1:
                        src = bass.AP(
                            tensor=cc4.tensor,
                            offset=cc4[:, :, 0, k].offset,
                            ap=[cc4.ap[0], [56, NT], [7, k + 1]],
                        )
                        dst = bass.AP(
                            tensor=cf.tensor,
                            offset=cf[:, :, k + 1, 0].offset,
                            ap=[cf.ap[0], [64, NT], [1, k + 1]],
                        )
                        nc.gpsimd.tensor_copy(dst, src)
                    if 7 - k >= 1:
                        src = bass.AP(
                            tensor=cc4.tensor,
                            offset=cc4[:, :, k + 1, k].offset,
                            ap=[cc4.ap[0], [56, NT], [7, 7 - k]],
                        )
                        dst = bass.AP(
                            tensor=cf.tensor,
                            offset=cf[:, :, k, k + 1].offset,
                            ap=[cf.ap[0], [64, NT], [1, 7 - k]],
                        )
                        nc.gpsimd.tensor_copy(dst, src)

            def diag_build(l, t):
                if t == 0:
                    _nop_read(nc.vector, cfull[l - 1][0:2, 0, 0, 0:2])
                # dgJ[p, half, j, col, i4]: per-(j,half) fully contiguous
                # [col,i] block so the mix-matmul rhs streams stride-1.
                dg = dgp.tile([P, 2, 8, P, 4], BF16, tag=f"dg{t % 2}")
                cf = cfull[l - 1]
                for half in range(2):
                    for j in range(8):
                        in0 = bass.AP(
                            tensor=cf.tensor,
                            offset=cf[:, t, j, 4 * half].offset,
                            ap=[cf.ap[0], [0, P], [1, 4]],
                        )
                        nc.vector.tensor_tensor(
                            dg[:, half, j], in0, identB, op=ALU.mult)
                return dg

            def mix_matmul(l, t, dg, z_in):
                """xfp[c][half] [P,512] f32 = x feature-major, accum over j."""
                D = DIMS[l]
                nch = D // 128
                xfps = []
                for c in range(nch):
                    for half in range(2):
                        xfp = pxf.tile([P, 512], F32, tag=f"xfp{c}{half}")
                        for j in range(8):
                            nc.tensor.matmul(
                                xfp,
                                z_in[:, t, j, c * 128 : (c + 1) * 128],
                                dg[:, half, j].rearrange("p a b -> p (a b)"),
                                start=(j == 0),
                                stop=(j == 7),
                            )
                        xfps.append(xfp)
                return xfps

            def evict_xf(l, t, xfps):
                """psum f32 -> sbuf bf16; spread across ACT/DVE/Pool."""
                nch = DIMS[l] // 128
                for idx, xfp in enumerate(xfps):
                    c, half = divmod(idx, 2)
                    dst = xfsb[:, c, half].rearrange("p a b -> p (a b)")
                    src_perm = bass.AP(
                        tensor=xfp.tensor, offset=xfp.offset,
                        ap=[xfp.ap[0], [1, 4], [4, P]],
                    )
                    if idx % 4 < 2:
                        nc.scalar.activation(dst, src_perm, AF.Copy)
                    else:
                        nc.vector.tensor_copy(dst, src_perm)

            # ================= layer 0 =================
            pg1 = pgp.tile([P, NT, 56], F32, tag="pg")
            for t in range(NT):
                if t > 0:
                    # collapse the xfp WAR (relu on ACT) into PE order
                    _nop_read(nc.tensor, z1[0:2, t - 1, :, 0:2])
                h0 = []
                for m in range(4):
                    hp = pxf.tile([P, 512], F32, tag=f"xfp{m // 2}{m % 2}")
                    nc.tensor.matmul(
                        hp, xf0[:, t], w0_sb[:, 2 * m : 2 * m + 2].rearrange(
                            "p a b -> p (a b)"),
                        start=True, stop=True,
                    )
                    h0.append(hp)
                h_aps = [h0[i // 2][:, (i % 2) * 256 : (i % 2) * 256 + 256]
                         for i in range(8)]
                ln_relu(0, t, h_aps, z1, 256)
                # feature-major z1 for gates (ACT-issued xbar transpose)
                nc.scalar.dma_start_transpose(
                    out=zf1[:, t], in_=z1[:, t].rearrange("p a b -> p (a b)")
                )
                # gates l1: accumulate 16 chunks into pg1[:, t]
                for c in range(16):
                    nc.tensor.matmul(
                        pg1[:, t],
                        zf1[:, t, c],
                        wgs1_sb[:, c],
                        start=(c == 0),
                        stop=(c == 15),
                    )
            coeff_prep(1, pg1)

            # ================= layer 1 =================
            pg2 = pgp.tile([P, NT, 56], F32, tag="pg")
            for t in range(NT):
                dg = diag_build(1, t)
                _nop_read(nc.tensor, dg[0:2, :, :, 0, 0:2])
                if t > 0:
                    _nop_read(nc.tensor, z2[0:2, t - 1, :, 0:2])
                xfps = mix_matmul(1, t, dg, z1)
                evict_xf(1, t, xfps)
                _nop_read(nc.tensor, xfsb[0:2, 0, :, 0, 0:2])
                _nop_read(nc.tensor, xfsb[0:2, 1, :, 0, 0:2])
                h = php.tile([P, 8, 128], F32, tag="ph")
                for i in range(8):
                    for c in range(2):
                        nc.tensor.matmul(
                            h[:, i],
                            xfsb[:, c, i // 4, i % 4],
                            w1_sb[:, i, c],
                            start=(c == 0),
                            stop=(c == 1),
                        )
                h_aps = [h[:, i] for i in range(8)]
                ln_relu(1, t, h_aps, z2, 128)
                nc.scalar.dma_start_transpose(
                    out=zf2[:, t], in_=z2[:, t].rearrange("p a b -> p (a b)")
                )
                for c in range(8):
                    nc.tensor.matmul(
                        pg2[:, t],
                        zf2[:, t, c],
                        wgs2_sb[:, c],
                        start=(c == 0),
                        stop=(c == 7),
                    )
            coeff_prep(2, pg2)

            # ================= layer 2 + head =================
            for t in range(NT):
                dg = diag_build(2, t)
                _nop_read(nc.tensor, dg[0:2, :, :, 0, 0:2])
                if t > 0:
                    _nop_read(nc.tensor, z3[0:2, t - 1, :, 0:2])
                xfps = mix_matmul(2, t, dg, z2)
                evict_xf(2, t, xfps)
                _nop_read(nc.tensor, xfsb[0:2, 0, 0, 0, 0:2])
                _nop_read(nc.tensor, xfsb[0:2, 0, 1, 0, 0:2])
                h = php.tile([P, 8, 128], F32, tag="ph")
                for i in range(8):
                    nc.tensor.matmul(
                        h[:, i, :64],
                        xfsb[:, 0, i // 4, i % 4],
                        w2_sb[:, i],
                        start=True,
                        stop=True,
                    )
                h_aps = [h[:, i, :64] for i in range(8)]
                ln_relu(2, t, h_aps, z3, 64)
                nc.scalar.dma_start_transpose(
                    out=z3f[:, t], in_=z3[:, t].rearrange("p a b -> p (a b)")
                )
                _nop_read(nc.tensor, z3f[0:2, t, :, 0:2])
                if t > 0:
                    _nop_read(nc.tensor, osb[0:2, t - 1, 0:2])
                po = pop.tile([P, 64], F32, tag="po")
                for c in range(4):
                    nc.tensor.matmul(
                        po[:, c * 16 : (c + 1) * 16],
                        z3f[:, t, c],
                        wo_sb[:, c],
                        start=True,
                        stop=True,
                    )
                nc.scalar.activation(osb[:, t], po, AF.Copy)
                osrc = bass.AP(
                    tensor=osb.tensor,
                    offset=osb[:, t].offset,
                    ap=[osb.ap[0], [16, 4], [8, 2], [1, 5]],
                )
                odst = bass.AP(
                    tensor=out_sb.tensor,
                    offset=out_sb[:, t].offset,
                    ap=[out_sb.ap[0], [10, 4], [5, 2], [1, 5]],
                )
                nc.scalar.activation(odst, osrc, AF.Tanh)

            scr = singles.tile([P, NT], F32)
            nc.gpsimd.tensor_copy(scr, out_sb[:, :, 39])
            nc.gpsimd.dma_start(
                out=out_ext.rearrange("(t p) f -> p t f", p=P), in_=out_sb
            )
    _split_excess_waits(nc)
    return nc


def _split_excess_waits(nc):
    """Hardware allows one sync wait per instruction. Move all-but-one wait
    of any multi-wait instruction onto same-engine NoOps inserted right
    before it (engine queue order makes this equivalent)."""
    m = nc.m
    for f in m.functions:
        for blk in f.blocks:
            i = 0
            while i < len(blk.instructions):
                ins = blk.instructions[i]
                si = ins.sync_info
                if si is not None and len(si.on_wait) > 1:
                    waits = list(si.on_wait)
                    for w in waits[:-1]:
                        nop = mybir.InstNoOp(
                            name=nc.get_next_instruction_name(),
                            bass_nofuse=True,
                        )
                        nop.engine = ins.engine
                        nop.sync_info = mybir.SyncInfo(
                            on_wait=[w], on_update=[])
                        blk.instructions.insert(i, nop)
                        i += 1
                    ins.sync_info = mybir.SyncInfo(
                        on_wait=[waits[-1]], on_update=list(si.on_update))
                i += 1


_NC_CACHE = None


def kernel(**inputs):
    global _NC_CACHE
    f32 = np.float32
    import ml_dtypes

    tobf = lambda x: np.asarray(x, dtype=f32).astype(ml_dtypes.bfloat16)

    Z = np.stack([np.asarray(inputs[f"Z{i}"], f32) for i in range(1, 9)], axis=1)
    gates = np.asarray(inputs["A"], f32)[:, GIDX]

    # ---- layer-0 attention on host ----
    Wg0 = np.asarray(inputs["Wg0"], f32)
    bg0 = np.asarray(inputs["bg0"], f32)
    Zn0 = Z[:, NEIGH, :]
    Zall0 = Zn0.reshape(B, 8, 21)
    logit0 = np.einsum("bif,ifo->bio", Zall0, Wg0) + bg0
    e0 = np.exp(logit0 - logit0.max(-1, keepdims=True))
    a0 = e0 / e0.sum(-1, keepdims=True)
    c0 = a0 * gates.reshape(B, 8, 7)
    x0 = Z + np.einsum("bik,bikd->bid", c0, Zn0)      # [B, 8, 3]
    x0p = np.zeros((B, 8, D0), f32)
    x0p[:, :, :3] = x0
    # feature-major: [32, B]
    x0f = x0p.reshape(B, 32).T.copy()

    DIMS = [3, 256, 128]
    wgs_list = []
    for l in range(1, 3):
        D = DIMS[l]
        Wg = np.asarray(inputs[f"Wg{l}"], f32)
        S = np.zeros((8 * D, 56), f32)
        for i in range(8):
            for k in range(7):
                j = int(NEIGH[i, k])
                S[j * D : (j + 1) * D, i * 7 : (i + 1) * 7] += Wg[i, k * D : (k + 1) * D, :]
        wgs_list.append(S)
    wgs1 = tobf(wgs_list[0].reshape(16, 128, 56))
    wgs2 = tobf(wgs_list[1].reshape(8, 128, 56))

    W0 = np.asarray(inputs["W0"], f32)              # [8,3,256]
    w0 = np.zeros((32, 8, 256), f32)
    for i in range(8):
        w0[D0 * i : D0 * i + 3, i, :] = W0[i]
    w1 = np.asarray(inputs["W1"], f32).reshape(8, 2, 128, 128)
    w2 = np.asarray(inputs["W2"], f32)
    Wo = np.asarray(inputs["Wout"], f32)
    wo = np.zeros((4, 128, 16), f32)
    for c in range(4):
        wo[c, 0:64, 0:5] = Wo[2 * c]
        wo[c, 64:128, 8:13] = Wo[2 * c + 1]

    identB = np.zeros((P, P, 4), f32)
    for p in range(P):
        identB[p, p, :] = 1.0

    if _NC_CACHE is None:
        _NC_CACHE = build_nc()
    nc = _NC_CACHE

    x0fb = tobf(x0f)
    in_maps = []
    for cix in range(NCORES):
        sl = slice(cix * BL, (cix + 1) * BL)
        in_maps.append({
            "x0f": x0fb[:, sl].copy(),
            "gates": gates[sl].copy(),
            "identB": tobf(identB.reshape(P, P * 4)),
            "wgs1": wgs1, "wgs2": wgs2,
            "w0": tobf(w0), "w1": tobf(w1), "w2": tobf(w2), "wo": tobf(wo),
        })
    res = run_bass_kernel_spmd(nc, in_maps, core_ids=list(range(NCORES)))
    outs = [res.results[c]["out"].reshape(BL, 8, 5) for c in range(NCORES)]
    return np.concatenate(outs, axis=0)


# revision 3
# speedup vs baseline: 1.0626x; 1.0626x over previous
"""Trainium2 kernel v2 for nn_ActorNetwork (8-branch graph-attention MLP).

Data-parallel over 8 cores (1024 samples each).  The neighbor mix
x_i = sum_j c~_ij z_j (per-sample coefficients) runs on the TensorEngine:
for each source branch j, matmul(lhsT = z_j sample-major chunk,
rhs = concat_i diag(c~_ij)) accumulates x feature-major in PSUM over j.
The diag matrices for a tile are built by ONE DVE tensor_tensor (2x_1P)
from a replicated-identity constant.  Feature-major z for the gate
matmuls comes from dma_start_transpose (ACT-issued, no PE/ACT copies).
LayerNorm stats via bn_stats off PSUM; relu+affine on ACT.
"""

import numpy as np

import concourse.bass as bass
import concourse.mybir as mybir
import concourse.tile as tile
from concourse.bass_utils import run_bass_kernel_spmd

B = 8192
NCORES = 8
BL = B // NCORES
P = 128
NT = BL // P
HID = [256, 128, 64]
OUT = 5
EPS = 1e-5
D0 = 4

NEIGH = np.array([[j for j in range(8) if j != i] for i in range(8)])
_G = [
    [(0, 1), (0, 2), (0, 3), (4, 0), (5, 0), (6, 0), (7, 0)],
    [(0, 1), (2, 1), (3, 1), (4, 1), (5, 1), (6, 1), (7, 1)],
    [(0, 2), (1, 2), (3, 2), (4, 2), (5, 2), (6, 2), (7, 2)],
    [(0, 3), (1, 3), (2, 3), (4, 3), (5, 3), (6, 3), (7, 3)],
    [(0, 4), (1, 4), (2, 4), (3, 4), (5, 4), (6, 4), (7, 4)],
    [(0, 5), (1, 5), (2, 5), (3, 5), (4, 5), (6, 5), (7, 5)],
    [(0, 6), (1, 6), (2, 6), (3, 6), (4, 6), (5, 6), (7, 6)],
    [(0, 7), (1, 7), (2, 7), (3, 7), (4, 7), (5, 7), (6, 7)],
]
GIDX = np.array([[r * 8 + c for (r, c) in row] for row in _G]).reshape(-1)

F32 = mybir.dt.float32
BF16 = mybir.dt.bfloat16
AF = mybir.ActivationFunctionType
ALU = mybir.AluOpType

# tail-drain split (see kernel.py of the prior session): one sync-wait per
# semaphore lane on SP nops so the final drain carries no waits.
_orig_dab = tile.TileContext._drain_and_barrier


def _patched_dab(self, tick_clock, wait_clock):
    from concourse.vector_clock import ScopedClock, VectorClock

    gc = tick_clock.global_clock
    nproc = len(list(gc))
    for proc in range(nproc):
        val = gc[proc]
        if val <= 0:
            continue
        v = VectorClock()
        v.require_at_least(proc, val)
        nop = self.nc.sync.nop(nofuse=True)
        wait_clock.add_sem_waits(nop.ins, ScopedClock({None: v}))
    self.nc.sync.drain()
    self.nc.all_engine_barrier()
    assert self.sems is not None
    popped = self.nc._tile_sem_poison_stack.pop()
    assert popped is self._sem_poison
    self.nc.clear_and_free_semaphores(list(self.sems.allocated().values()))
    self.nc.all_engine_barrier()


tile.TileContext._drain_and_barrier = _patched_dab


def _bcast_last(ap, n):
    return bass.AP(tensor=ap.tensor, offset=ap.offset, ap=[*ap.ap, [0, n]])


def _nop_read(eng, ap):
    """Engine NOP carrying a read-AP: absorbs one foreign sem lane into the
    engine's program order so the next real instruction needs <=1 wait.
    The AP must be attached before add_instruction so the tile scheduler
    sees the dependency (it is stripped again at lowering)."""
    ins = mybir.InstNoOp(
        name=eng.bass.get_next_instruction_name(),
        bass_nofuse=True,
        ins=[eng.lower_ap(ap)],
    )
    return eng.add_instruction(ins)


def build_nc():
    nc = bass.Bass()
    DIMS = [D0, 256, 128]
    ECOLS = [256, 128, 64]

    x0f_ext = nc.declare_dram_parameter("x0f", [32, BL], BF16, isOutput=False)
    gates_ext = nc.declare_dram_parameter("gates", [BL, 56], F32, isOutput=False)
    identB_ext = nc.declare_dram_parameter("identB", [P, P * 4], BF16, isOutput=False)
    wgs1_ext = nc.declare_dram_parameter("wgs1", [16, 128, 56], BF16, isOutput=False)
    wgs2_ext = nc.declare_dram_parameter("wgs2", [8, 128, 56], BF16, isOutput=False)
    w0_ext = nc.declare_dram_parameter("w0", [32, 8, 256], BF16, isOutput=False)
    w1_ext = nc.declare_dram_parameter("w1", [8, 2, 128, 128], BF16, isOutput=False)
    w2_ext = nc.declare_dram_parameter("w2", [8, 128, 64], BF16, isOutput=False)
    wo_ext = nc.declare_dram_parameter("wo", [4, 128, 16], BF16, isOutput=False)
    out_ext = nc.declare_dram_parameter("out", [BL, 40], F32, isOutput=True)

    with tile.TileContext(nc) as tc:
        with (
            tc.tile_pool(name="singles", bufs=1) as singles,
            tc.tile_pool(name="dg", bufs=1) as dgp,
            tc.tile_pool(name="pxf", bufs=1, space="PSUM") as pxf,
            tc.tile_pool(name="ph", bufs=1, space="PSUM") as php,
            tc.tile_pool(name="pg", bufs=1, space="PSUM") as pgp,
            tc.tile_pool(name="po", bufs=1, space="PSUM") as pop,
        ):
            # ---------- constants / weights ----------
            eps_t = singles.tile([P, 1], F32)
            nc.vector.memset(eps_t, EPS)

            xf0 = singles.tile([32, NT, P], BF16)
            nc.gpsimd.dma_start(
                out=xf0, in_=x0f_ext.rearrange("f (t p) -> f t p", p=P)
            )
            w0_sb = singles.tile([32, 8, 256], BF16)
            nc.gpsimd.dma_start(out=w0_sb, in_=w0_ext.rearrange("p i f -> p i f"))
            wgs1_sb = singles.tile([128, 16, 56], BF16)
            nc.gpsimd.dma_start(out=wgs1_sb, in_=wgs1_ext.rearrange("k p f -> p k f"))
            gates_sb = singles.tile([P, NT, 56], F32)
            nc.gpsimd.dma_start(
                out=gates_sb, in_=gates_ext.rearrange("(t p) f -> p t f", p=P)
            )
            identB = singles.tile([P, P, 4], BF16)
            nc.gpsimd.dma_start(
                out=identB.rearrange("p a b -> p (a b)"), in_=identB_ext[:, :]
            )
            wgs2_sb = singles.tile([128, 8, 56], BF16)
            nc.gpsimd.dma_start(out=wgs2_sb, in_=wgs2_ext.rearrange("k p f -> p k f"))
            w1_sb = singles.tile([128, 8, 2, 128], BF16)
            nc.gpsimd.dma_start(out=w1_sb, in_=w1_ext.rearrange("i k p f -> p i k f"))
            w2_sb = singles.tile([128, 8, 64], BF16)
            nc.gpsimd.dma_start(out=w2_sb, in_=w2_ext.rearrange("i p f -> p i f"))
            wo_sb = singles.tile([128, 4, 16], BF16)
            nc.gpsimd.dma_start(out=wo_sb, in_=wo_ext.rearrange("c p f -> p c f"))

            # ---------- activations ----------
            z1 = singles.tile([P, NT, 8, 256], BF16)
            z2 = singles.tile([P, NT, 8, 128], BF16)
            z3 = singles.tile([P, NT, 8, 64], BF16)
            zf1 = singles.tile([P, NT, 16, P], BF16)
            zf2 = singles.tile([P, NT, 8, P], BF16)
            z3f = singles.tile([P, NT, 4, P], BF16)
            xfsb = singles.tile([P, 2, 2, 4, P], BF16)   # [chunk, half, i4, q]

            e_sb = singles.tile([P, NT, 56], F32)
            cc4 = singles.tile([P, NT, 8, 7], BF16)
            cfull0 = singles.tile([P, NT, 8, 8], BF16)
            cfull1 = singles.tile([P, NT, 8, 8], BF16)
            cfull = [cfull0, cfull1]
            s8 = singles.tile([P, NT, 8], F32)
            sr8 = singles.tile([P, NT, 8], F32)
            stats = singles.tile([P, NT, 8, 6], F32)
            t1v = singles.tile([P, NT, 8], F32)
            t2v = singles.tile([P, NT, 8], F32)
            t3v = singles.tile([P, NT, 8], F32)
            varv = singles.tile([P, NT, 8], F32)
            stdv = singles.tile([P, NT, 8], F32)
            rstd = singles.tile([P, NT, 8], F32)
            nmr = singles.tile([P, NT, 8], F32)
            scr_a = singles.tile([2, 2], F32)
            osb = singles.tile([P, NT, 64], F32)
            out_sb = singles.tile([P, NT, 40], F32)

            for l in range(2):
                nc.vector.memset(cfull[l], 1.0)

            def ln_relu(l, t, h_aps, z_out, E):
                """h_aps: list of 8 psum APs [P, E] (branch-major). bn_stats
                + baseline LN chain + 8 ACT relus into z_out[:, t]."""
                for i in range(8):
                    nc.vector.bn_stats(stats[:, t, i], h_aps[i])
                sA = stats[:, t, :, 1]
                sB = stats[:, t, :, 4]
                sC = stats[:, t, :, 2]
                sD = stats[:, t, :, 5]
                nc.vector.tensor_tensor(t1v[:, t], sA, sB, op=ALU.add)
                nc.vector.tensor_tensor(t2v[:, t], sC, sD, op=ALU.add)
                nc.vector.tensor_tensor(t3v[:, t], sA, sB, op=ALU.subtract)
                nc.vector.tensor_tensor(t3v[:, t], t3v[:, t], t3v[:, t], op=ALU.mult)
                nc.vector.tensor_scalar_mul(t2v[:, t], t2v[:, t], 1.0 / E)
                nc.vector.scalar_tensor_tensor(
                    varv[:, t], t3v[:, t], 0.25, t2v[:, t],
                    op0=ALU.mult, op1=ALU.add,
                )
                nc.scalar.activation(
                    stdv[:, t], varv[:, t], AF.Sqrt, bias=eps_t[:, 0:1]
                )
                nc.vector.reciprocal(rstd[:, t], stdv[:, t])
                nc.vector.scalar_tensor_tensor(
                    nmr[:, t], t1v[:, t], -0.5, rstd[:, t],
                    op0=ALU.mult, op1=ALU.mult,
                )
                nc.scalar.activation(scr_a, nmr[0:2, t, 0:2], AF.Copy)
                for i in range(8):
                    nc.scalar.activation(
                        z_out[:, t, i],
                        h_aps[i],
                        AF.Relu,
                        bias=nmr[:, t, i : i + 1],
                        scale=rstd[:, t, i : i + 1],
                    )

            def coeff_prep(l, pg_ps):
                """exp + softmax + gate scaling + scatter into cfull[l-1]."""
                # absorb prior-layer DVE reads of e_sb into ACT, and prior
                # gpsimd cfull writes into DVE, so Exp/TT carry <=1 wait
                if l == 2:
                    _nop_read(nc.scalar, cc4[0:2, 0, 0, 0:2])
                    _nop_read(nc.vector, cfull[0][0:2, 0, 0, 0:2])
                nc.scalar.activation(e_sb, pg_ps, AF.Exp)
                e4 = e_sb.rearrange("p t (a b) -> p t a b", a=8)
                nc.vector.tensor_reduce(
                    s8, e4, axis=mybir.AxisListType.X, op=ALU.add
                )
                nc.vector.reciprocal(
                    sr8.rearrange("p t a -> p (t a)"),
                    s8.rearrange("p t a -> p (t a)"),
                )
                # cc4 = e * gates * (1/s)  (bf16 out on the second op)
                nc.vector.tensor_tensor(e_sb, e_sb, gates_sb, op=ALU.mult)
                nc.vector.tensor_tensor(
                    cc4,
                    e_sb.rearrange("p t (a b) -> p t a b", a=8),
                    _bcast_last(sr8, 7),
                    op=ALU.mult,
                )
                # scatter (i,k) -> cfull[p, t, j, i] on gpsimd (14 rects)
                cf = cfull[l - 1]
                for k in range(7):
                    if k + 1 >= 1:
                        src = bass.AP(
                            tensor=cc4.tensor,
                            offset=cc4[:, :, 0, k].offset,
                            ap=[cc4.ap[0], [56, NT], [7, k + 1]],
                        )
                        dst = bass.AP(
                            tensor=cf.tensor,
                            offset=cf[:, :, k + 1, 0].offset,
                            ap=[cf.ap[0], [64, NT], [1, k + 1]],
                        )
                        nc.gpsimd.tensor_copy(dst, src)
                    if 7 - k >= 1:
                        src = bass.AP(
                            tensor=cc4.tensor,
                            offset=cc4[:, :, k + 1, k].offset,
                            ap=[cc4.ap[0], [56, NT], [7, 7 - k]],
                        )
                        dst = bass.AP(
                            tensor=cf.tensor,
                            offset=cf[:, :, k, k + 1].offset,
                            ap=[cf.ap[0], [64, NT], [1, 7 - k]],
                        )
                        nc.gpsimd.tensor_copy(dst, src)

            def diag_build(l, t):
                if t == 0:
                    _nop_read(nc.vector, cfull[l - 1][0:2, 0, 0, 0:2])
                # dgJ[p, half, j, col, i4]: per-(j,half) fully contiguous
                # [col,i] block so the mix-matmul rhs streams stride-1.
                dg = dgp.tile([P, 2, 8, P, 4], BF16, tag=f"dg{t % 2}")
                cf = cfull[l - 1]
                for half in range(2):
                    for j in range(8):
                        in0 = bass.AP(
                            tensor=cf.tensor,
                            offset=cf[:, t, j, 4 * half].offset,
                            ap=[cf.ap[0], [0, P], [1, 4]],
                        )
                        nc.vector.tensor_tensor(
                            dg[:, half, j], in0, identB, op=ALU.mult)
                return dg

            def mix_matmul(l, t, dg, z_in):
                """xfp[c][half] [P,512] f32 = x feature-major, accum over j."""
                D = DIMS[l]
                nch = D // 128
                xfps = []
                for c in range(nch):
                    for half in range(2):
                        xfp = pxf.tile([P, 512], F32, tag=f"xfp{c}{half}")
                        for j in range(8):
                            nc.tensor.matmul(
                                xfp,
                                z_in[:, t, j, c * 128 : (c + 1) * 128],
                                dg[:, half, j].rearrange("p a b -> p (a b)"),
                                start=(j == 0),
                                stop=(j == 7),
                            )
                        xfps.append(xfp)
                return xfps

            def evict_xf(l, t, xfps):
                """psum f32 -> sbuf bf16; spread across ACT/DVE/Pool."""
                nch = DIMS[l] // 128
                for idx, xfp in enumerate(xfps):
                    c, half = divmod(idx, 2)
                    dst = xfsb[:, c, half].rearrange("p a b -> p (a b)")
                    src_perm = bass.AP(
                        tensor=xfp.tensor, offset=xfp.offset,
                        ap=[xfp.ap[0], [1, 4], [4, P]],
                    )
                    if idx % 4 < 2:
                        nc.scalar.activation(dst, src_perm, AF.Copy)
                    else:
                        nc.vector.tensor_copy(dst, src_perm)

            # ================= layer 0 =================
            pg1 = pgp.tile([P, NT, 56], F32, tag="pg")
            for t in range(NT):
                if t > 0:
                    # collapse the xfp WAR (relu on ACT) into PE order
                    _nop_read(nc.tensor, z1[0:2, t - 1, :, 0:2])
                h0 = []
                for m in range(4):
                    hp = pxf.tile([P, 512], F32, tag=f"xfp{m // 2}{m % 2}")
                    nc.tensor.matmul(
                        hp, xf0[:, t], w0_sb[:, 2 * m : 2 * m + 2].rearrange(
                            "p a b -> p (a b)"),
                        start=True, stop=True,
                    )
                    h0.append(hp)
                h_aps = [h0[i // 2][:, (i % 2) * 256 : (i % 2) * 256 + 256]
                         for i in range(8)]
                ln_relu(0, t, h_aps, z1, 256)
                # feature-major z1 for gates (ACT-issued xbar transpose)
                nc.scalar.dma_start_transpose(
                    out=zf1[:, t], in_=z1[:, t].rearrange("p a b -> p (a b)")
                )
                # gates l1: accumulate 16 chunks into pg1[:, t]
                for c in range(16):
                    nc.tensor.matmul(
                        pg1[:, t],
                        zf1[:, t, c],
                        wgs1_sb[:, c],
                        start=(c == 0),
                        stop=(c == 15),
                    )
            coeff_prep(1, pg1)

            # ================= layer 1 =================
            pg2 = pgp.tile([P, NT, 56], F32, tag="pg")
            dg_next = diag_build(1, 0)
            for t in range(NT):
                dg = dg_next
                _nop_read(nc.tensor, dg[0:2, :, :, 0, 0:2])
                if t > 0:
                    _nop_read(nc.tensor, z2[0:2, t - 1, :, 0:2])
                xfps = mix_matmul(1, t, dg, z1)
                if t + 1 < NT:
                    dg_next = diag_build(1, t + 1)
                evict_xf(1, t, xfps)
                _nop_read(nc.tensor, xfsb[0:2, 0, :, 0, 0:2])
                _nop_read(nc.tensor, xfsb[0:2, 1, :, 0, 0:2])
                h = php.tile([P, 8, 128], F32, tag="ph")
                for i in range(8):
                    for c in range(2):
                        nc.tensor.matmul(
                            h[:, i],
                            xfsb[:, c, i // 4, i % 4],
                            w1_sb[:, i, c],
                            start=(c == 0),
                            stop=(c == 1),
                        )
                h_aps = [h[:, i] for i in range(8)]
                ln_relu(1, t, h_aps, z2, 128)
                nc.scalar.dma_start_transpose(
                    out=zf2[:, t], in_=z2[:, t].rearrange("p a b -> p (a b)")
                )
                for c in range(8):
                    nc.tensor.matmul(
                        pg2[:, t],
                        zf2[:, t, c],
                        wgs2_sb[:, c],
                        start=(c == 0),
                        stop=(c == 7),
                    )
            coeff_prep(2, pg2)

            # ================= layer 2 + head =================
            dg_next = diag_build(2, 0)
            for t in range(NT):
                dg = dg_next
                _nop_read(nc.tensor, dg[0:2, :, :, 0, 0:2])
                if t > 0:
                    _nop_read(nc.tensor, z3[0:2, t - 1, :, 0:2])
                xfps = mix_matmul(2, t, dg, z2)
                if t + 1 < NT:
                    dg_next = diag_build(2, t + 1)
                evict_xf(2, t, xfps)
                _nop_read(nc.tensor, xfsb[0:2, 0, 0, 0, 0:2])
                _nop_read(nc.tensor, xfsb[0:2, 0, 1, 0, 0:2])
                h = php.tile([P, 8, 128], F32, tag="ph")
                for i in range(8):
                    nc.tensor.matmul(
                        h[:, i, :64],
                        xfsb[:, 0, i // 4, i % 4],
                        w2_sb[:, i],
                        start=True,
                        stop=True,
                    )
                h_aps = [h[:, i, :64] for i in range(8)]
                ln_relu(2, t, h_aps, z3, 64)
                nc.scalar.dma_start_transpose(
                    out=z3f[:, t], in_=z3[:, t].rearrange("p a b -> p (a b)")
                )
                _nop_read(nc.tensor, z3f[0:2, t, :, 0:2])
                if t > 0:
                    _nop_read(nc.tensor, osb[0:2, t - 1, 0:2])
                po = pop.tile([P, 64], F32, tag="po")
                for c in range(4):
                    nc.tensor.matmul(
                        po[:, c * 16 : (c + 1) * 16],
                        z3f[:, t, c],
                        wo_sb[:, c],
                        start=True,
                        stop=True,
                    )
                nc.scalar.activation(osb[:, t], po, AF.Copy)
                osrc = bass.AP(
                    tensor=osb.tensor,
                    offset=osb[:, t].offset,
                    ap=[osb.ap[0], [16, 4], [8, 2], [1, 5]],
                )
                odst = bass.AP(
                    tensor=out_sb.tensor,
                    offset=out_sb[:, t].offset,
                    ap=[out_sb.ap[0], [10, 4], [5, 2], [1, 5]],
                )
                nc.scalar.activation(odst, osrc, AF.Tanh)

            scr = singles.tile([P, NT], F32)
            nc.gpsimd.tensor_copy(scr, out_sb[:, :, 39])
            nc.gpsimd.dma_start(
                out=out_ext.rearrange("(t p) f -> p t f", p=P), in_=out_sb
            )
    _split_excess_waits(nc)
    return nc


def _split_excess_waits(nc):
    """Hardware allows one sync wait per instruction. Move all-but-one wait
    of any multi-wait instruction onto same-engine NoOps inserted right
    before it (engine queue order makes this equivalent)."""
    m = nc.m
    for f in m.functions:
        for blk in f.blocks:
            i = 0
            while i < len(blk.instructions):
                ins = blk.instructions[i]
                si = ins.sync_info
                if si is not None and len(si.on_wait) > 1:
                    waits = list(si.on_wait)
                    for w in waits[:-1]:
                        nop = mybir.InstNoOp(
                            name=nc.get_next_instruction_name(),
                            bass_nofuse=True,
                        )
                        nop.engine = ins.engine
                        nop.sync_info = mybir.SyncInfo(
                            on_wait=[w], on_update=[])
                        blk.instructions.insert(i, nop)
                        i += 1
                    ins.sync_info = mybir.SyncInfo(
                        on_wait=[waits[-1]], on_update=list(si.on_update))
                i += 1


_NC_CACHE = None


def kernel(**inputs):
    global _NC_CACHE
    f32 = np.float32
    import ml_dtypes

    tobf = lambda x: np.asarray(x, dtype=f32).astype(ml_dtypes.bfloat16)

    Z = np.stack([np.asarray(inputs[f"Z{i}"], f32) for i in range(1, 9)], axis=1)
    gates = np.asarray(inputs["A"], f32)[:, GIDX]

    # ---- layer-0 attention on host ----
    Wg0 = np.asarray(inputs["Wg0"], f32)
    bg0 = np.asarray(inputs["bg0"], f32)
    Zn0 = Z[:, NEIGH, :]
    Zall0 = Zn0.reshape(B, 8, 21)
    logit0 = np.einsum("bif,ifo->bio", Zall0, Wg0) + bg0
    e0 = np.exp(logit0 - logit0.max(-1, keepdims=True))
    a0 = e0 / e0.sum(-1, keepdims=True)
    c0 = a0 * gates.reshape(B, 8, 7)
    x0 = Z + np.einsum("bik,bikd->bid", c0, Zn0)      # [B, 8, 3]
    x0p = np.zeros((B, 8, D0), f32)
    x0p[:, :, :3] = x0
    # feature-major: [32, B]
    x0f = x0p.reshape(B, 32).T.copy()

    DIMS = [3, 256, 128]
    wgs_list = []
    for l in range(1, 3):
        D = DIMS[l]
        Wg = np.asarray(inputs[f"Wg{l}"], f32)
        S = np.zeros((8 * D, 56), f32)
        for i in range(8):
            for k in range(7):
                j = int(NEIGH[i, k])
                S[j * D : (j + 1) * D, i * 7 : (i + 1) * 7] += Wg[i, k * D : (k + 1) * D, :]
        wgs_list.append(S)
    wgs1 = tobf(wgs_list[0].reshape(16, 128, 56))
    wgs2 = tobf(wgs_list[1].reshape(8, 128, 56))

    W0 = np.asarray(inputs["W0"], f32)              # [8,3,256]
    w0 = np.zeros((32, 8, 256), f32)
    for i in range(8):
        w0[D0 * i : D0 * i + 3, i, :] = W0[i]
    w1 = np.asarray(inputs["W1"], f32).reshape(8, 2, 128, 128)
    w2 = np.asarray(inputs["W2"], f32)
    Wo = np.asarray(inputs["Wout"], f32)
    wo = np.zeros((4, 128, 16), f32)
    for c in range(4):
        wo[c, 0:64, 0:5] = Wo[2 * c]
        wo[c, 64:128, 8:13] = Wo[2 * c + 1]

    identB = np.zeros((P, P, 4), f32)
    for p in range(P):
        identB[p, p, :] = 1.0

    if _NC_CACHE is None:
        _NC_CACHE = build_nc()
    nc = _NC_CACHE

    x0fb = tobf(x0f)
    in_maps = []
    for cix in range(NCORES):
        sl = slice(cix * BL, (cix + 1) * BL)
        in_maps.append({
            "x0f": x0fb[:, sl].copy(),
            "gates": gates[sl].copy(),
            "identB": tobf(identB.reshape(P, P * 4)),
            "wgs1": wgs1, "wgs2": wgs2,
            "w0": tobf(w0), "w1": tobf(w1), "w2": tobf(w2), "wo": tobf(wo),
        })
    res = run_bass_kernel_spmd(nc, in_maps, core_ids=list(range(NCORES)))
    outs = [res.results[c]["out"].reshape(BL, 8, 5) for c in range(NCORES)]
    return np.concatenate(outs, axis=0)


# revision 4
# speedup vs baseline: 1.0729x; 1.0097x over previous
"""Trainium2 kernel v2 for nn_ActorNetwork (8-branch graph-attention MLP).

Data-parallel over 8 cores (1024 samples each).  The neighbor mix
x_i = sum_j c~_ij z_j (per-sample coefficients) runs on the TensorEngine:
for each source branch j, matmul(lhsT = z_j sample-major chunk,
rhs = concat_i diag(c~_ij)) accumulates x feature-major in PSUM over j.
The diag matrices for a tile are built by ONE DVE tensor_tensor (2x_1P)
from a replicated-identity constant.  Feature-major z for the gate
matmuls comes from dma_start_transpose (ACT-issued, no PE/ACT copies).
LayerNorm stats via bn_stats off PSUM; relu+affine on ACT.
"""

import numpy as np

import concourse.bass as bass
import concourse.mybir as mybir
import concourse.tile as tile
from concourse.bass_utils import run_bass_kernel_spmd

B = 8192
NCORES = 8
BL = B // NCORES
P = 128
NT = BL // P
HID = [256, 128, 64]
OUT = 5
EPS = 1e-5
D0 = 4

NEIGH = np.array([[j for j in range(8) if j != i] for i in range(8)])
_G = [
    [(0, 1), (0, 2), (0, 3), (4, 0), (5, 0), (6, 0), (7, 0)],
    [(0, 1), (2, 1), (3, 1), (4, 1), (5, 1), (6, 1), (7, 1)],
    [(0, 2), (1, 2), (3, 2), (4, 2), (5, 2), (6, 2), (7, 2)],
    [(0, 3), (1, 3), (2, 3), (4, 3), (5, 3), (6, 3), (7, 3)],
    [(0, 4), (1, 4), (2, 4), (3, 4), (5, 4), (6, 4), (7, 4)],
    [(0, 5), (1, 5), (2, 5), (3, 5), (4, 5), (6, 5), (7, 5)],
    [(0, 6), (1, 6), (2, 6), (3, 6), (4, 6), (5, 6), (7, 6)],
    [(0, 7), (1, 7), (2, 7), (3, 7), (4, 7), (5, 7), (6, 7)],
]
GIDX = np.array([[r * 8 + c for (r, c) in row] for row in _G]).reshape(-1)

F32 = mybir.dt.float32
BF16 = mybir.dt.bfloat16
AF = mybir.ActivationFunctionType
ALU = mybir.AluOpType

# tail-drain split (see kernel.py of the prior session): one sync-wait per
# semaphore lane on SP nops so the final drain carries no waits.
_orig_dab = tile.TileContext._drain_and_barrier


def _patched_dab(self, tick_clock, wait_clock):
    from concourse.vector_clock import ScopedClock, VectorClock

    gc = tick_clock.global_clock
    nproc = len(list(gc))
    for proc in range(nproc):
        val = gc[proc]
        if val <= 0:
            continue
        v = VectorClock()
        v.require_at_least(proc, val)
        nop = self.nc.sync.nop(nofuse=True)
        wait_clock.add_sem_waits(nop.ins, ScopedClock({None: v}))
    self.nc.sync.drain()
    self.nc.all_engine_barrier()
    assert self.sems is not None
    popped = self.nc._tile_sem_poison_stack.pop()
    assert popped is self._sem_poison
    self.nc.clear_and_free_semaphores(list(self.sems.allocated().values()))
    self.nc.all_engine_barrier()


tile.TileContext._drain_and_barrier = _patched_dab


def _bcast_last(ap, n):
    return bass.AP(tensor=ap.tensor, offset=ap.offset, ap=[*ap.ap, [0, n]])


def _nop_read(eng, ap):
    """Engine NOP carrying a read-AP: absorbs one foreign sem lane into the
    engine's program order so the next real instruction needs <=1 wait.
    The AP must be attached before add_instruction so the tile scheduler
    sees the dependency (it is stripped again at lowering)."""
    ins = mybir.InstNoOp(
        name=eng.bass.get_next_instruction_name(),
        bass_nofuse=True,
        ins=[eng.lower_ap(ap)],
    )
    return eng.add_instruction(ins)


def build_nc():
    nc = bass.Bass()
    DIMS = [D0, 256, 128]
    ECOLS = [256, 128, 64]

    x0f_ext = nc.declare_dram_parameter("x0f", [32, BL], BF16, isOutput=False)
    gates_ext = nc.declare_dram_parameter("gates", [BL, 56], F32, isOutput=False)
    identB_ext = nc.declare_dram_parameter("identB", [P, P * 4], BF16, isOutput=False)
    wgs1_ext = nc.declare_dram_parameter("wgs1", [16, 128, 56], BF16, isOutput=False)
    wgs2_ext = nc.declare_dram_parameter("wgs2", [8, 128, 56], BF16, isOutput=False)
    w0_ext = nc.declare_dram_parameter("w0", [32, 8, 256], BF16, isOutput=False)
    w1_ext = nc.declare_dram_parameter("w1", [8, 2, 128, 128], BF16, isOutput=False)
    w2_ext = nc.declare_dram_parameter("w2", [8, 128, 64], BF16, isOutput=False)
    wo_ext = nc.declare_dram_parameter("wo", [4, 128, 16], BF16, isOutput=False)
    out_ext = nc.declare_dram_parameter("out", [BL, 40], F32, isOutput=True)

    with tile.TileContext(nc) as tc:
        with (
            tc.tile_pool(name="singles", bufs=1) as singles,
            tc.tile_pool(name="dg", bufs=1) as dgp,
            tc.tile_pool(name="pxf", bufs=1, space="PSUM") as pxf,
            tc.tile_pool(name="ph", bufs=1, space="PSUM") as php,
            tc.tile_pool(name="pg", bufs=1, space="PSUM") as pgp,
            tc.tile_pool(name="po", bufs=1, space="PSUM") as pop,
        ):
            # ---------- constants / weights ----------
            eps_t = singles.tile([P, 1], F32)
            nc.vector.memset(eps_t, EPS)

            xf0 = singles.tile([32, NT, P], BF16)
            nc.gpsimd.dma_start(
                out=xf0, in_=x0f_ext.rearrange("f (t p) -> f t p", p=P)
            )
            w0_sb = singles.tile([32, 8, 256], BF16)
            nc.gpsimd.dma_start(out=w0_sb, in_=w0_ext.rearrange("p i f -> p i f"))
            wgs1_sb = singles.tile([128, 16, 56], BF16)
            nc.gpsimd.dma_start(out=wgs1_sb, in_=wgs1_ext.rearrange("k p f -> p k f"))
            gates_sb = singles.tile([P, NT, 56], F32)
            nc.gpsimd.dma_start(
                out=gates_sb, in_=gates_ext.rearrange("(t p) f -> p t f", p=P)
            )
            identB = singles.tile([P, P, 4], BF16)
            nc.gpsimd.dma_start(
                out=identB.rearrange("p a b -> p (a b)"), in_=identB_ext[:, :]
            )
            wgs2_sb = singles.tile([128, 8, 56], BF16)
            nc.gpsimd.dma_start(out=wgs2_sb, in_=wgs2_ext.rearrange("k p f -> p k f"))
            w1_sb = singles.tile([128, 8, 2, 128], BF16)
            nc.gpsimd.dma_start(out=w1_sb, in_=w1_ext.rearrange("i k p f -> p i k f"))
            w2_sb = singles.tile([128, 8, 64], BF16)
            nc.gpsimd.dma_start(out=w2_sb, in_=w2_ext.rearrange("i p f -> p i f"))
            wo_sb = singles.tile([128, 4, 16], BF16)
            nc.gpsimd.dma_start(out=wo_sb, in_=wo_ext.rearrange("c p f -> p c f"))

            # ---------- activations ----------
            z1 = singles.tile([P, NT, 8, 256], BF16)
            z2 = singles.tile([P, NT, 8, 128], BF16)
            z3 = singles.tile([P, NT, 8, 64], BF16)
            zf1 = singles.tile([P, NT, 16, P], BF16)
            zf2 = singles.tile([P, NT, 8, P], BF16)
            z3f = singles.tile([P, NT, 4, P], BF16)
            xfsb = singles.tile([P, 2, 2, 4, P], BF16)   # [chunk, half, i4, q]

            e_sb = singles.tile([P, NT, 56], F32)
            cc4 = singles.tile([P, NT, 8, 7], BF16)
            cfull0 = singles.tile([P, NT, 8, 8], BF16)
            cfull1 = singles.tile([P, NT, 8, 8], BF16)
            cfull = [cfull0, cfull1]
            s8 = singles.tile([P, NT, 8], F32)
            sr8 = singles.tile([P, NT, 8], F32)
            stats = singles.tile([P, NT, 8, 6], F32)
            t1v = singles.tile([P, NT, 8], F32)
            t2v = singles.tile([P, NT, 8], F32)
            t3v = singles.tile([P, NT, 8], F32)
            varv = singles.tile([P, NT, 8], F32)
            stdv = singles.tile([P, NT, 8], F32)
            rstd = singles.tile([P, NT, 8], F32)
            nmr = singles.tile([P, NT, 8], F32)
            scr_a = singles.tile([2, 2], F32)
            osb = singles.tile([P, NT, 64], F32)
            out_sb = singles.tile([P, NT, 40], F32)

            for l in range(2):
                nc.vector.memset(cfull[l], 1.0)

            def ln_relu(l, t, h_aps, z_out, E):
                """h_aps: list of 8 psum APs [P, E] (branch-major). bn_stats
                + baseline LN chain + 8 ACT relus into z_out[:, t]."""
                for i in range(8):
                    nc.vector.bn_stats(stats[:, t, i], h_aps[i])
                sA = stats[:, t, :, 1]
                sB = stats[:, t, :, 4]
                sC = stats[:, t, :, 2]
                sD = stats[:, t, :, 5]
                nc.vector.tensor_tensor(t1v[:, t], sA, sB, op=ALU.add)
                nc.vector.tensor_tensor(t2v[:, t], sC, sD, op=ALU.add)
                nc.vector.tensor_tensor(t3v[:, t], sA, sB, op=ALU.subtract)
                nc.vector.tensor_tensor(t3v[:, t], t3v[:, t], t3v[:, t], op=ALU.mult)
                nc.vector.tensor_scalar_mul(t2v[:, t], t2v[:, t], 1.0 / E)
                nc.vector.scalar_tensor_tensor(
                    varv[:, t], t3v[:, t], 0.25, t2v[:, t],
                    op0=ALU.mult, op1=ALU.add,
                )
                nc.scalar.activation(
                    stdv[:, t], varv[:, t], AF.Sqrt, bias=eps_t[:, 0:1]
                )
                nc.vector.reciprocal(rstd[:, t], stdv[:, t])
                nc.vector.scalar_tensor_tensor(
                    nmr[:, t], t1v[:, t], -0.5, rstd[:, t],
                    op0=ALU.mult, op1=ALU.mult,
                )
                nc.scalar.activation(scr_a, nmr[0:2, t, 0:2], AF.Copy)
                for i in range(8):
                    nc.scalar.activation(
                        z_out[:, t, i],
                        h_aps[i],
                        AF.Relu,
                        bias=nmr[:, t, i : i + 1],
                        scale=rstd[:, t, i : i + 1],
                    )

            def coeff_prep(l, pg_ps, t0=0, t1=NT):
                """exp + softmax + gate scaling + scatter into cfull[l-1]
                for tiles t0..t1 (half-calls overlap the previous layer)."""
                nt = t1 - t0
                if l == 2 and t0 == 0:
                    _nop_read(nc.scalar, cc4[0:2, 0, 0, 0:2])
                    _nop_read(nc.vector, cfull[0][0:2, 0, 0, 0:2])
                ts = slice(t0, t1)
                nc.scalar.activation(e_sb[:, ts], pg_ps[:, ts], AF.Exp)
                e4 = e_sb.rearrange("p t (a b) -> p t a b", a=8)
                nc.vector.tensor_reduce(
                    s8[:, ts], e4[:, ts], axis=mybir.AxisListType.X, op=ALU.add
                )
                nc.vector.reciprocal(
                    s8[:, ts].rearrange("p t a -> p (t a)"),
                    s8[:, ts].rearrange("p t a -> p (t a)"),
                )
                # cc4 = e * gates * (1/s)  (bf16 out on the second op)
                nc.vector.tensor_tensor(
                    e_sb[:, ts], e_sb[:, ts], gates_sb[:, ts], op=ALU.mult)
                nc.vector.tensor_tensor(
                    cc4[:, ts],
                    e4[:, ts],
                    _bcast_last(s8[:, ts], 7),
                    op=ALU.mult,
                )
                # scatter (i,k) -> cfull[p, t, j, i] on gpsimd (14 rects)
                cf = cfull[l - 1]
                for k in range(7):
                    if k + 1 >= 1:
                        src = bass.AP(
                            tensor=cc4.tensor,
                            offset=cc4[:, t0, 0, k].offset,
                            ap=[cc4.ap[0], [56, nt], [7, k + 1]],
                        )
                        dst = bass.AP(
                            tensor=cf.tensor,
                            offset=cf[:, t0, k + 1, 0].offset,
                            ap=[cf.ap[0], [64, nt], [1, k + 1]],
                        )
                        nc.gpsimd.tensor_copy(dst, src)
                    if 7 - k >= 1:
                        src = bass.AP(
                            tensor=cc4.tensor,
                            offset=cc4[:, t0, k + 1, k].offset,
                            ap=[cc4.ap[0], [56, nt], [7, 7 - k]],
                        )
                        dst = bass.AP(
                            tensor=cf.tensor,
                            offset=cf[:, t0, k, k + 1].offset,
                            ap=[cf.ap[0], [64, nt], [1, 7 - k]],
                        )
                        nc.gpsimd.tensor_copy(dst, src)

            def diag_build(l, t):
                if t == 0:
                    _nop_read(nc.vector, cfull[l - 1][0:2, 0, 0, 0:2])
                # dgJ[p, half, j, col, i4]: per-(j,half) fully contiguous
                # [col,i] block so the mix-matmul rhs streams stride-1.
                dg = dgp.tile([P, 2, 8, P, 4], BF16, tag=f"dg{t % 2}")
                cf = cfull[l - 1]
                for half in range(2):
                    for j in range(8):
                        in0 = bass.AP(
                            tensor=cf.tensor,
                            offset=cf[:, t, j, 4 * half].offset,
                            ap=[cf.ap[0], [0, P], [1, 4]],
                        )
                        nc.vector.tensor_tensor(
                            dg[:, half, j], in0, identB, op=ALU.mult)
                return dg

            def mix_matmul(l, t, dg, z_in):
                """xfp[c][half] [P,512] f32 = x feature-major, accum over j."""
                D = DIMS[l]
                nch = D // 128
                xfps = []
                for c in range(nch):
                    for half in range(2):
                        xfp = pxf.tile([P, 512], F32, tag=f"xfp{c}{half}")
                        for j in range(8):
                            nc.tensor.matmul(
                                xfp,
                                z_in[:, t, j, c * 128 : (c + 1) * 128],
                                dg[:, half, j].rearrange("p a b -> p (a b)"),
                                start=(j == 0),
                                stop=(j == 7),
                            )
                        xfps.append(xfp)
                return xfps

            def evict_xf(l, t, xfps):
                """psum f32 -> sbuf bf16; spread across ACT/DVE/Pool."""
                nch = DIMS[l] // 128
                for idx, xfp in enumerate(xfps):
                    c, half = divmod(idx, 2)
                    dst = xfsb[:, c, half].rearrange("p a b -> p (a b)")
                    src_perm = bass.AP(
                        tensor=xfp.tensor, offset=xfp.offset,
                        ap=[xfp.ap[0], [1, 4], [4, P]],
                    )
                    if idx % 4 < 2:
                        nc.scalar.activation(dst, src_perm, AF.Copy)
                    else:
                        nc.vector.tensor_copy(dst, src_perm)

            # ================= layer 0 =================
            pg1 = pgp.tile([P, NT, 56], F32, tag="pg")
            for t in range(NT):
                if t == 5:
                    coeff_prep(1, pg1, 0, 4)
                if t > 0:
                    # collapse the xfp WAR (relu on ACT) into PE order
                    _nop_read(nc.tensor, z1[0:2, t - 1, :, 0:2])
                h0 = []
                for m in range(4):
                    hp = pxf.tile([P, 512], F32, tag=f"xfp{m // 2}{m % 2}")
                    nc.tensor.matmul(
                        hp, xf0[:, t], w0_sb[:, 2 * m : 2 * m + 2].rearrange(
                            "p a b -> p (a b)"),
                        start=True, stop=True,
                    )
                    h0.append(hp)
                h_aps = [h0[i // 2][:, (i % 2) * 256 : (i % 2) * 256 + 256]
                         for i in range(8)]
                ln_relu(0, t, h_aps, z1, 256)
                # feature-major z1 for gates (ACT-issued xbar transpose)
                nc.scalar.dma_start_transpose(
                    out=zf1[:, t], in_=z1[:, t].rearrange("p a b -> p (a b)")
                )
                # gates l1: accumulate 16 chunks into pg1[:, t]
                for c in range(16):
                    nc.tensor.matmul(
                        pg1[:, t],
                        zf1[:, t, c],
                        wgs1_sb[:, c],
                        start=(c == 0),
                        stop=(c == 15),
                    )
            coeff_prep(1, pg1, 4, NT)

            # ================= layer 1 =================
            pg2 = pgp.tile([P, NT, 56], F32, tag="pg")
            dg_next = diag_build(1, 0)
            for t in range(NT):
                if t == 5:
                    coeff_prep(2, pg2, 0, 4)
                dg = dg_next
                _nop_read(nc.tensor, dg[0:2, :, :, 0, 0:2])
                if t > 0:
                    _nop_read(nc.tensor, z2[0:2, t - 1, :, 0:2])
                xfps = mix_matmul(1, t, dg, z1)
                if t + 1 < NT:
                    dg_next = diag_build(1, t + 1)
                evict_xf(1, t, xfps)
                _nop_read(nc.tensor, xfsb[0:2, 0, :, 0, 0:2])
                _nop_read(nc.tensor, xfsb[0:2, 1, :, 0, 0:2])
                h = php.tile([P, 8, 128], F32, tag="ph")
                for i in range(8):
                    for c in range(2):
                        nc.tensor.matmul(
                            h[:, i],
                            xfsb[:, c, i // 4, i % 4],
                            w1_sb[:, i, c],
                            start=(c == 0),
                            stop=(c == 1),
                        )
                h_aps = [h[:, i] for i in range(8)]
                ln_relu(1, t, h_aps, z2, 128)
                nc.scalar.dma_start_transpose(
                    out=zf2[:, t], in_=z2[:, t].rearrange("p a b -> p (a b)")
                )
                for c in range(8):
                    nc.tensor.matmul(
                        pg2[:, t],
                        zf2[:, t, c],
                        wgs2_sb[:, c],
                        start=(c == 0),
                        stop=(c == 7),
                    )
            coeff_prep(2, pg2, 4, NT)

            # ================= layer 2 + head =================
            dg_next = diag_build(2, 0)
            for t in range(NT):
                dg = dg_next
                _nop_read(nc.tensor, dg[0:2, :, :, 0, 0:2])
                if t > 0:
                    _nop_read(nc.tensor, z3[0:2, t - 1, :, 0:2])
                xfps = mix_matmul(2, t, dg, z2)
                if t + 1 < NT:
                    dg_next = diag_build(2, t + 1)
                evict_xf(2, t, xfps)
                _nop_read(nc.tensor, xfsb[0:2, 0, 0, 0, 0:2])
                _nop_read(nc.tensor, xfsb[0:2, 0, 1, 0, 0:2])
                h = php.tile([P, 8, 128], F32, tag="ph")
                for i in range(8):
                    nc.tensor.matmul(
                        h[:, i, :64],
                        xfsb[:, 0, i // 4, i % 4],
                        w2_sb[:, i],
                        start=True,
                        stop=True,
                    )
                h_aps = [h[:, i, :64] for i in range(8)]
                ln_relu(2, t, h_aps, z3, 64)
                nc.scalar.dma_start_transpose(
                    out=z3f[:, t], in_=z3[:, t].rearrange("p a b -> p (a b)")
                )
                _nop_read(nc.tensor, z3f[0:2, t, :, 0:2])
                if t > 0:
                    _nop_read(nc.tensor, osb[0:2, t - 1, 0:2])
                po = pop.tile([P, 64], F32, tag="po")
                for c in range(4):
                    nc.tensor.matmul(
                        po[:, c * 16 : (c + 1) * 16],
                        z3f[:, t, c],
                        wo_sb[:, c],
                        start=True,
                        stop=True,
                    )
                nc.scalar.activation(osb[:, t], po, AF.Copy)
                osrc = bass.AP(
                    tensor=osb.tensor,
                    offset=osb[:, t].offset,
                    ap=[osb.ap[0], [16, 4], [8, 2], [1, 5]],
                )
                odst = bass.AP(
                    tensor=out_sb.tensor,
                    offset=out_sb[:, t].offset,
                    ap=[out_sb.ap[0], [10, 4], [5, 2], [1, 5]],
                )
                nc.scalar.activation(odst, osrc, AF.Tanh)

            scr = singles.tile([P, NT], F32)
            nc.gpsimd.tensor_copy(scr, out_sb[:, :, 39])
            nc.gpsimd.dma_start(
                out=out_ext.rearrange("(t p) f -> p t f", p=P), in_=out_sb
            )
    _split_excess_waits(nc)
    return nc


def _split_excess_waits(nc):
    """Hardware allows one sync wait per instruction. Move all-but-one wait
    of any multi-wait instruction onto same-engine NoOps inserted right
    before it (engine queue order makes this equivalent)."""
    m = nc.m
    for f in m.functions:
        for blk in f.blocks:
            i = 0
            while i < len(blk.instructions):
                ins = blk.instructions[i]
                si = ins.sync_info
                if si is not None and len(si.on_wait) > 1:
                    waits = list(si.on_wait)
                    for w in waits[:-1]:
                        nop = mybir.InstNoOp(
                            name=nc.get_next_instruction_name(),
                            bass_nofuse=True,
                        )
                        nop.engine = ins.engine
                        nop.sync_info = mybir.SyncInfo(
                            on_wait=[w], on_update=[])
                        blk.instructions.insert(i, nop)
                        i += 1
                    ins.sync_info = mybir.SyncInfo(
                        on_wait=[waits[-1]], on_update=list(si.on_update))
                i += 1


_NC_CACHE = None


def kernel(**inputs):
    global _NC_CACHE
    f32 = np.float32
    import ml_dtypes

    tobf = lambda x: np.asarray(x, dtype=f32).astype(ml_dtypes.bfloat16)

    Z = np.stack([np.asarray(inputs[f"Z{i}"], f32) for i in range(1, 9)], axis=1)
    gates = np.asarray(inputs["A"], f32)[:, GIDX]

    # ---- layer-0 attention on host ----
    Wg0 = np.asarray(inputs["Wg0"], f32)
    bg0 = np.asarray(inputs["bg0"], f32)
    Zn0 = Z[:, NEIGH, :]
    Zall0 = Zn0.reshape(B, 8, 21)
    logit0 = np.einsum("bif,ifo->bio", Zall0, Wg0) + bg0
    e0 = np.exp(logit0 - logit0.max(-1, keepdims=True))
    a0 = e0 / e0.sum(-1, keepdims=True)
    c0 = a0 * gates.reshape(B, 8, 7)
    x0 = Z + np.einsum("bik,bikd->bid", c0, Zn0)      # [B, 8, 3]
    x0p = np.zeros((B, 8, D0), f32)
    x0p[:, :, :3] = x0
    # feature-major: [32, B]
    x0f = x0p.reshape(B, 32).T.copy()

    DIMS = [3, 256, 128]
    wgs_list = []
    for l in range(1, 3):
        D = DIMS[l]
        Wg = np.asarray(inputs[f"Wg{l}"], f32)
        S = np.zeros((8 * D, 56), f32)
        for i in range(8):
            for k in range(7):
                j = int(NEIGH[i, k])
                S[j * D : (j + 1) * D, i * 7 : (i + 1) * 7] += Wg[i, k * D : (k + 1) * D, :]
        wgs_list.append(S)
    wgs1 = tobf(wgs_list[0].reshape(16, 128, 56))
    wgs2 = tobf(wgs_list[1].reshape(8, 128, 56))

    W0 = np.asarray(inputs["W0"], f32)              # [8,3,256]
    w0 = np.zeros((32, 8, 256), f32)
    for i in range(8):
        w0[D0 * i : D0 * i + 3, i, :] = W0[i]
    w1 = np.asarray(inputs["W1"], f32).reshape(8, 2, 128, 128)
    w2 = np.asarray(inputs["W2"], f32)
    Wo = np.asarray(inputs["Wout"], f32)
    wo = np.zeros((4, 128, 16), f32)
    for c in range(4):
        wo[c, 0:64, 0:5] = Wo[2 * c]
        wo[c, 64:128, 8:13] = Wo[2 * c + 1]

    identB = np.zeros((P, P, 4), f32)
    for p in range(P):
        identB[p, p, :] = 1.0

    if _NC_CACHE is None:
        _NC_CACHE = build_nc()
    nc = _NC_CACHE

    x0fb = tobf(x0f)
    in_maps = []
    for cix in range(NCORES):
        sl = slice(cix * BL, (cix + 1) * BL)
        in_maps.append({
            "x0f": x0fb[:, sl].copy(),
            "gates": gates[sl].copy(),
            "identB": tobf(identB.reshape(P, P * 4)),
            "wgs1": wgs1, "wgs2": wgs2,
            "w0": tobf(w0), "w1": tobf(w1), "w2": tobf(w2), "wo": tobf(wo),
        })
    res = run_bass_kernel_spmd(nc, in_maps, core_ids=list(range(NCORES)))
    outs = [res.results[c]["out"].reshape(BL, 8, 5) for c in range(NCORES)]
    return np.concatenate(outs, axis=0)
